# revision 28
# baseline (speedup 1.0000x reference)
"""CFNO forward kernel for Trainium2 (8 NeuronCores, data-parallel over batch).

The reference computes, per 16x16 patch p (flattened to 256):
    fft = FFT_256(p) (ortho); fc = fft @ Wc^T + bc; y = Re(IFFT_16(fc)) (ortho)
    z = y @ conv_w^T + conv_b;  out = GroupNorm_8(z) * gamma + beta

Because p is real and every step before GroupNorm is linear, the whole chain
folds into one real matrix on the host:
    M2 = Re(F @ Wc^T @ G) @ conv_w^T   [256, 16]
    b2 = Re(bc @ G) @ conv_w^T + conv_b [16]
    z  = p @ M2 + b2
(F = symmetric 256-pt DFT matrix / sqrt(256); G = inverse 16-pt DFT / sqrt(16))

On-device per core (one batch image, x [2048, 2048]):
  - 16 row-blocks of 128 image rows; SBUF layout [128 part=(hblk, s1), 2048]
  - per block, 16 PSUM-accumulating matmuls (one per patch-column offset s2,
    free dim 128) with a block-diagonal lhsT so all 8 h-blocks share a matmul;
    float32r keeps full fp32 storage with a fast (TF32-like) PE mode
  - z stays in PSUM (4 banks hold all 16 blocks); bn_stats reads PSUM; the
    fc bias b2 is folded into the final normalize coefficients
  - one mask-matmul does the grouped cross-partition reduce AND broadcast
  - normalize (z*A + B) fused with the PSUM->SBUF move, chunked, with the
    output DMA of each chunk overlapping the next chunk's normalize

DMA scheduling (probed on HW): only full 128-partition transfers run at
line rate — partition-subset dma_starts (e.g. [0:112] + tails) drop to
~half the per-engine rate (port/engine misalignment), and non-divisible
partition counts like [0:127] degenerate onto a single engine.  So every
block is one uniform [128, 2048] dma_start (~417 GB/s marginal when
pipelined); the ~17%-slow engine 15 (known HW quirk) is left as-is.
"""

import numpy as np
from contextlib import ExitStack

CHUNK = 16
GROUPS = 8
EPS = 1e-5
B, C, H, W = 8, 1, 2048, 2048
D = 16
D_IN = CHUNK * CHUNK * C  # 256
HP = H // CHUNK  # 128 patch rows
WP = W // CHUNK  # 128 patch cols
P = 128
RB = 16  # 128-row blocks per image
N_CORES = 8

_CACHED_NC = {}


def _build_nc(mm_dtype="float32r"):
    import concourse.bass as bass
    import concourse.tile as tile
    from concourse import bacc, mybir

    f32 = mybir.dt.float32
    mmdt = getattr(mybir.dt, mm_dtype)
    nc = bacc.Bacc("TRN2", target_bir_lowering=False, debug=False,
                   num_devices=N_CORES)

    x = nc.dram_tensor("x", [H, W], mmdt, kind="ExternalInput").ap()
    # compact weights: [32, 0:128] replication lhsT + [32, 128:640] pre-placed
    # rhs; one matmul broadcasts/places the per-hblk blocks (80KB vs 1MB)
    wlc = nc.dram_tensor("wlc", [32, 640], mmdt, kind="ExternalInput").ap()
    gmask = nc.dram_tensor("gmask", [P, P], f32, kind="ExternalInput").ap()
    consts = nc.dram_tensor("consts", [P, 3], f32, kind="ExternalInput").ap()
    # [p=(hblk,e), rg, w] flattened (rg = hi//8); host reorders to [D, HP, WP]
    out = nc.dram_tensor("out", [P, RB * WP], f32, kind="ExternalOutput").ap()

    Ident = mybir.ActivationFunctionType.Identity
    Sqrt = mybir.ActivationFunctionType.Sqrt
    Mult = mybir.AluOpType.mult
    Add = mybir.AluOpType.add
    Sub = mybir.AluOpType.subtract

    with tile.TileContext(nc) as tc, ExitStack() as ctx:
        const_pool = ctx.enter_context(tc.tile_pool(name="const", bufs=1))
        # all 16 x blocks fit in SBUF: the stream is never WAR-gated
        xin2 = ctx.enter_context(tc.tile_pool(name="xin2", bufs=8))
        zpool = ctx.enter_context(tc.tile_pool(name="z", bufs=1))
        # 4 persistent PSUM banks hold z for all 16 blocks; 1 more for gp
        zpsum = ctx.enter_context(tc.tile_pool(name="zp", bufs=4, space="PSUM"))
        psg = ctx.enter_context(tc.tile_pool(name="psg", bufs=1, space="PSUM"))

        # x row-block rb covers image rows [rb*128, (rb+1)*128):
        # row = rb*128 + p, p = (hblk, s1)
        xr = x.rearrange("(rb p) c -> rb p c", rb=RB, p=P)

        # compact weights (80KB instead of 1MB off the stream): one matmul
        # (contraction 32) broadcasts each hblk's 16x16 block, already placed
        # at sub-offset (hb%2)*16 of a 32-wide frame; then 4 copies at the
        # legal 32-aligned partition bases move it into the block-diagonal
        # lhsT.  Zeros come from an f32 staging memset (f32r memset is not a
        # valid ISA instruction).
        ws = const_pool.tile([32, 640], mmdt)
        nc.sync.dma_start(out=ws, in_=wlc)
        gmt = const_pool.tile([P, P], f32)
        nc.scalar.dma_start(out=gmt, in_=gmask)
        cvt = const_pool.tile([P, 3], f32)
        nc.scalar.dma_start(out=cvt, in_=consts)
        wtile = const_pool.tile([P, CHUNK, P], mmdt)
        wflat = wtile.rearrange("p s m -> p (s m)")
        zstage = const_pool.tile([P, 512], f32)
        nc.vector.memset(zstage, 0.0)
        for k in range(4):
            nc.vector.tensor_copy(wflat[:, 512 * k:512 * (k + 1)], zstage)
        wexp = psg.tile([P, 512], f32, name="wexp")
        nc.tensor.matmul(wexp, lhsT=ws[:, 0:128], rhs=ws[:, 128:640],
                         start=True, stop=True)
        for k in range(4):
            nc.vector.tensor_copy(
                wtile[32 * k:32 * (k + 1), :, 32 * k:32 * (k + 1)],
                wexp[32 * k:32 * (k + 1)].rearrange("p (a b) -> p a b",
                                                    a=CHUNK))
        epst = const_pool.tile([P, 1], f32)
        nc.vector.memset(epst, EPS)
        # touch Sqrt (exact scale/bias shape used later) early so its
        # ACT table loads during the stream, not in the stats chain
        warm = const_pool.tile([P, 1], f32)
        nc.scalar.activation(out=warm, in_=epst, func=Sqrt,
                             bias=epst, scale=-1.0)
        nc.scalar.activation(out=warm, in_=epst, func=Ident,
                             bias=epst, scale=epst)

        # stream: x blocks on the SP ring, two blocks per SBUF tile (the
        # matmul then runs N=256, hiding the ~213ns contended LDWEIGHTS).
        # The last two blocks are single tiles: block 15's matmuls run
        # post-stream at the uncontended rate, shortening the tail.
        xts = []  # (tile, q, first block)
        for t in range(8):
            xt = xin2.tile([P, 2, W], mmdt, tag="xt2", name=f"xt{t}")
            # transposed view -> per-partition 8KB descs round-robined over
            # engines; measured ~20% faster per engine than the contiguous
            # 8-partition-chunk spray
            nc.sync.dma_start(out=xt, in_=xr[2 * t:2 * t + 2]
                              .transpose([1, 0, 2]))
            xts.append((xt.rearrange("p q (w s) -> p q w s", s=CHUNK), 2,
                        2 * t))

        # bn_stats per PSUM bank-half-or-more (never a region the PE still
        # writes): banks 0-2 in one 512-elem call, bank 3 split 2+2 so the
        # tail only waits for the last 256-elem call
        assert nc.vector.BN_STATS_FMAX >= 4 * WP
        statsall = zpool.tile([P, 5, nc.vector.BN_STATS_DIM], f32)
        zts = [zpsum.tile([P, 4, WP], f32, tag="zt", name=f"zt{g}")
               for g in range(4)]

        for xs, q, b0 in xts:
            pt = zts[b0 // 4][:, b0 % 4:b0 % 4 + q]
            for s2 in range(CHUNK):
                nc.tensor.matmul(pt, lhsT=wtile[:, s2, :],
                                 rhs=xs[:, :, :, s2],
                                 start=(s2 == 0), stop=(s2 == CHUNK - 1))
            done = b0 + q  # blocks completed so far
            if done % 4 == 0 and done <= 12:
                g = done // 4 - 1
                nc.vector.bn_stats(out=statsall[:, g],
                                   in_=zts[g].rearrange("p a b -> p (a b)"))
            elif done == 14:
                nc.vector.bn_stats(out=statsall[:, 3],
                                   in_=zts[3][:, 0:2].rearrange(
                                       "p a b -> p (a b)"))
            elif done == 16:
                nc.vector.bn_stats(out=statsall[:, 4],
                                   in_=zts[3][:, 2:4].rearrange(
                                       "p a b -> p (a b)"))

        # Per-partition raw mean'/var over all 2048 elements; fc bias b2 is
        # folded in here (z_true = raw + b2) and into the normalize offset.
        mv = zpool.tile([P, 2], f32)
        nc.vector.bn_aggr(out=mv, in_=statsall)
        # me2 = (-mean, E2) of biased z per partition:
        #   negmean = -(mean' + b2);  E2 = negmean^2 + var
        me2 = zpool.tile([P, 2], f32)
        nc.vector.tensor_scalar(out=me2[:, 0:1], in0=mv[:, 0:1],
                                scalar1=cvt[:, 0:1], scalar2=-1.0,
                                op0=Add, op1=Mult)
        nc.vector.scalar_tensor_tensor(
            out=me2[:, 1:2], in0=me2[:, 0:1], scalar=me2[:, 0:1],
            in1=mv[:, 1:2], op0=Mult, op1=Add)
        # Grouped cross-partition average + broadcast in one matmul:
        # gp[p'] = (1/16) * sum_{p in group(p')} me2[p] = (-mean_g, E2_g)
        gp = psg.tile([P, 2], f32)
        nc.tensor.matmul(gp, lhsT=gmt, rhs=me2, start=True, stop=True)
        gsb = zpool.tile([P, 2], f32)
        nc.vector.tensor_copy(gsb, gp)
        # negvar = mean_g^2 - E2_g;  sd = sqrt(-negvar + eps)
        negvar = zpool.tile([P, 1], f32)
        nc.vector.scalar_tensor_tensor(
            out=negvar, in0=gsb[:, 0:1], scalar=gsb[:, 0:1], in1=gsb[:, 1:2],
            op0=Mult, op1=Sub)
        sd = zpool.tile([P, 1], f32)
        nc.scalar.activation(out=sd, in_=negvar, func=Sqrt, bias=epst,
                             scale=-1.0)
        # v = b2 - mean_g (overlaps with Sqrt on the ACT engine)
        v = zpool.tile([P, 1], f32)
        nc.vector.tensor_add(v, cvt[:, 0:1], gsb[:, 0:1])
        rs = zpool.tile([P, 1], f32)
        nc.vector.reciprocal(rs, sd)
        # out = raw*A + B2 with A = rsqrt*gamma, B2 = (b2 - mean_g)*A + beta
        A = zpool.tile([P, 1], f32)
        nc.vector.tensor_mul(A, rs, cvt[:, 1:2])
        B2 = zpool.tile([P, 1], f32)
        nc.vector.scalar_tensor_tensor(
            out=B2, in0=A, scalar=v, in1=cvt[:, 2:3], op0=Mult, op1=Add)

        # normalize PSUM->SBUF in tapered chunks (small first so the output
        # DMA starts early; ACT and DVE alternate), each chunk's DMA
        # overlapping the next chunk's normalize
        onorm = zpool.tile([P, RB * WP], f32)
        chunks = [(0, 1), (1, 4), (4, 8), (8, 12), (12, 16)]
        for ci, (b0, b1) in enumerate(chunks):
            sl = slice(b0 * WP, b1 * WP)
            g = b0 // 4
            zin = zts[g][:, b0 % 4:b0 % 4 + (b1 - b0)].rearrange(
                "p a b -> p (a b)")
            if ci % 2 == 0:
                nc.vector.tensor_scalar(out=onorm[:, sl], in0=zin,
                                        scalar1=A, scalar2=B2,
                                        op0=Mult, op1=Add)
            else:
                nc.scalar.activation(out=onorm[:, sl], in_=zin,
                                     func=Ident, scale=A, bias=B2)
            eng = nc.sync if ci % 2 == 0 else nc.scalar
            eng.dma_start(out=out[:, sl], in_=onorm[:, sl])

    nc.compile()
    return nc


def _host_weights(fc_wr, fc_wi, fc_br, fc_bi, conv_w, conv_b, gamma, beta):
    fc_wr = np.asarray(fc_wr, np.float64)
    fc_wi = np.asarray(fc_wi, np.float64)
    fc_br = np.asarray(fc_br, np.float64)
    fc_bi = np.asarray(fc_bi, np.float64)
    conv_w = np.asarray(conv_w, np.float64)
    conv_b = np.asarray(conv_b, np.float64)
    gamma = np.asarray(gamma, np.float64)
    beta = np.asarray(beta, np.float64)

    j = np.arange(D_IN)
    F = np.exp(-2j * np.pi * np.outer(j, j) / D_IN) / np.sqrt(D_IN)
    d = np.arange(D)
    G = np.exp(2j * np.pi * np.outer(d, d) / D) / np.sqrt(D)
    Wc = fc_wr + 1j * fc_wi
    bc = fc_br + 1j * fc_bi
    M2 = (np.real(F @ Wc.T @ G) @ conv_w.T).astype(np.float32)  # [256, 16]
    b2 = (np.real(bc @ G) @ conv_w.T + conv_b).astype(np.float32)  # [16]

    # Compact weights, contraction dim k=(par, s1) of 32:
    #   lhsT cols [0:128]: wlc[par*16+s1, p] = (p%16==s1 and (p//16)%2==par)
    #   rhs  cols [128:640] ([s2, 32-frame]): the 16x16 M2 block at
    #     sub-offset 16*par, so out[(hb,s1'), (s2,c)] lands pre-placed for
    #     the 32-aligned on-device copies.
    wlc = np.zeros((32, 640), np.float32)
    p_ = np.arange(P)
    for par in range(2):
        for s1 in range(CHUNK):
            k = par * 16 + s1
            wlc[k, :P] = ((p_ % 16 == s1) & ((p_ // 16) % 2 == par))
            for s2 in range(CHUNK):
                c0 = 128 + s2 * 32 + 16 * par
                wlc[k, c0:c0 + 16] = M2[s1 * 16 + s2]

    # Group-average + broadcast mask; each partition holds 2048 elements,
    # each group spans 16 partitions -> scale 1/16 on the per-partition means
    pidx = np.arange(P)
    grp = (pidx % D) // (D // GROUPS)
    gmask = (grp[:, None] == grp[None, :]).astype(np.float32) / 16.0

    e = pidx % D
    consts = np.stack([b2[e], gamma.astype(np.float32)[e],
                       beta.astype(np.float32)[e]], axis=1)  # [128, 3]
    return wlc, gmask, consts


def kernel(x, fc_wr, fc_wi, fc_br, fc_bi, conv_w, conv_b, gamma, beta,
           _return_results=False, _trace=False, _mm_dtype="float32r"):
    from concourse.bass_utils import run_bass_kernel_spmd

    if _mm_dtype not in _CACHED_NC:
        _CACHED_NC[_mm_dtype] = _build_nc(_mm_dtype)
    nc = _CACHED_NC[_mm_dtype]

    wlc, gmask, consts = _host_weights(fc_wr, fc_wi, fc_br, fc_bi,
                                      conv_w, conv_b, gamma, beta)
    x = np.ascontiguousarray(np.asarray(x, np.float32).reshape(B, H, W))
    in_maps = [{"x": x[b], "wlc": wlc, "gmask": gmask, "consts": consts}
               for b in range(N_CORES)]
    res = run_bass_kernel_spmd(nc, in_maps, list(range(N_CORES)),
                               trace=_trace)
    # device layout [p=(hblk,e), rg, w] -> [D, HP, WP], hi = rg*8 + hblk
    out = np.stack(
        [res.results[b]["out"].reshape(8, D, RB, WP)
         .transpose(1, 2, 0, 3).reshape(D, HP, WP)
         for b in range(N_CORES)], axis=0)
    if _return_results:
        return out, res
    return out


# revision 29
# speedup vs baseline: 1.0625x; 1.0625x over previous
"""CFNO forward kernel for Trainium2 (8 NeuronCores, data-parallel over batch).

The reference computes, per 16x16 patch p (flattened to 256):
    fft = FFT_256(p) (ortho); fc = fft @ Wc^T + bc; y = Re(IFFT_16(fc)) (ortho)
    z = y @ conv_w^T + conv_b;  out = GroupNorm_8(z) * gamma + beta

Because p is real and every step before GroupNorm is linear, the whole chain
folds into one real matrix on the host:
    M2 = Re(F @ Wc^T @ G) @ conv_w^T   [256, 16]
    b2 = Re(bc @ G) @ conv_w^T + conv_b [16]
    z  = p @ M2 + b2
(F = symmetric 256-pt DFT matrix / sqrt(256); G = inverse 16-pt DFT / sqrt(16))

On-device per core (one batch image, x [2048, 2048]):
  - 16 row-blocks of 128 image rows; SBUF layout [128 part=(hblk, s1), 2048]
  - per block, 16 PSUM-accumulating matmuls (one per patch-column offset s2,
    free dim 128) with a block-diagonal lhsT so all 8 h-blocks share a matmul;
    float32r keeps full fp32 storage with a fast (TF32-like) PE mode
  - z stays in PSUM (4 banks hold all 16 blocks); bn_stats reads PSUM; the
    fc bias b2 is folded into the final normalize coefficients
  - one mask-matmul does the grouped cross-partition reduce AND broadcast
  - normalize (z*A + B) fused with the PSUM->SBUF move, chunked, with the
    output DMA of each chunk overlapping the next chunk's normalize

DMA scheduling (probed on HW): only full 128-partition transfers run at
line rate — partition-subset dma_starts (e.g. [0:112] + tails) drop to
~half the per-engine rate (port/engine misalignment), and non-divisible
partition counts like [0:127] degenerate onto a single engine.  So every
block is one uniform [128, 2048] dma_start (~417 GB/s marginal when
pipelined); the ~17%-slow engine 15 (known HW quirk) is left as-is.
"""

import numpy as np
from contextlib import ExitStack

CHUNK = 16
GROUPS = 8
EPS = 1e-5
B, C, H, W = 8, 1, 2048, 2048
D = 16
D_IN = CHUNK * CHUNK * C  # 256
HP = H // CHUNK  # 128 patch rows
WP = W // CHUNK  # 128 patch cols
P = 128
RB = 16  # 128-row blocks per image
N_CORES = 8

_CACHED_NC = {}


def _build_nc(mm_dtype="float32r"):
    import concourse.bass as bass
    import concourse.tile as tile
    from concourse import bacc, mybir

    f32 = mybir.dt.float32
    mmdt = getattr(mybir.dt, mm_dtype)
    nc = bacc.Bacc("TRN2", target_bir_lowering=False, debug=False,
                   num_devices=N_CORES)

    x = nc.dram_tensor("x", [H, W], mmdt, kind="ExternalInput").ap()
    # compact weights: [32, 0:128] replication lhsT + [32, 128:640] pre-placed
    # rhs; one matmul broadcasts/places the per-hblk blocks (80KB vs 1MB)
    wlc = nc.dram_tensor("wlc", [32, 640], mmdt, kind="ExternalInput").ap()
    gmask = nc.dram_tensor("gmask", [P, P], f32, kind="ExternalInput").ap()
    consts = nc.dram_tensor("consts", [P, 3], f32, kind="ExternalInput").ap()
    # [p=(hblk,e), rg, w] flattened (rg = hi//8); host reorders to [D, HP, WP]
    out = nc.dram_tensor("out", [P, RB * WP], f32, kind="ExternalOutput").ap()

    Ident = mybir.ActivationFunctionType.Identity
    Sqrt = mybir.ActivationFunctionType.Sqrt
    Mult = mybir.AluOpType.mult
    Add = mybir.AluOpType.add
    Sub = mybir.AluOpType.subtract

    with tile.TileContext(nc) as tc, ExitStack() as ctx:
        const_pool = ctx.enter_context(tc.tile_pool(name="const", bufs=1))
        # all 16 x blocks fit in SBUF: the stream is never WAR-gated
        xin2 = ctx.enter_context(tc.tile_pool(name="xin2", bufs=8))
        zpool = ctx.enter_context(tc.tile_pool(name="z", bufs=1))
        # 4 persistent PSUM banks hold z for all 16 blocks; 1 more for gp
        zpsum = ctx.enter_context(tc.tile_pool(name="zp", bufs=4, space="PSUM"))
        psg = ctx.enter_context(tc.tile_pool(name="psg", bufs=1, space="PSUM"))

        # x row-block rb covers image rows [rb*128, (rb+1)*128):
        # row = rb*128 + p, p = (hblk, s1)
        xr = x.rearrange("(rb p) c -> rb p c", rb=RB, p=P)

        # compact weights (80KB instead of 1MB off the stream): one matmul
        # (contraction 32) broadcasts each hblk's 16x16 block, already placed
        # at sub-offset (hb%2)*16 of a 32-wide frame; then 4 copies at the
        # legal 32-aligned partition bases move it into the block-diagonal
        # lhsT.  Zeros come from an f32 staging memset (f32r memset is not a
        # valid ISA instruction).
        ws = const_pool.tile([32, 640], mmdt)
        nc.sync.dma_start(out=ws, in_=wlc)
        gmt = const_pool.tile([P, P], f32)
        nc.scalar.dma_start(out=gmt, in_=gmask)
        cvt = const_pool.tile([P, 3], f32)
        nc.scalar.dma_start(out=cvt, in_=consts)
        wtile = const_pool.tile([P, CHUNK, P], mmdt)
        wflat = wtile.rearrange("p s m -> p (s m)")
        zstage = const_pool.tile([P, 512], f32)
        nc.vector.memset(zstage, 0.0)
        for k in range(4):
            nc.vector.tensor_copy(wflat[:, 512 * k:512 * (k + 1)], zstage)
        wexp = psg.tile([P, 512], f32, name="wexp")
        nc.tensor.matmul(wexp, lhsT=ws[:, 0:128], rhs=ws[:, 128:640],
                         start=True, stop=True)
        for k in range(4):
            nc.vector.tensor_copy(
                wtile[32 * k:32 * (k + 1), :, 32 * k:32 * (k + 1)],
                wexp[32 * k:32 * (k + 1)].rearrange("p (a b) -> p a b",
                                                    a=CHUNK))
        epst = const_pool.tile([P, 1], f32)
        nc.vector.memset(epst, EPS)
        # touch Sqrt (exact scale/bias shape used later) early so its
        # ACT table loads during the stream, not in the stats chain
        warm = const_pool.tile([P, 1], f32)
        nc.scalar.activation(out=warm, in_=epst, func=Sqrt,
                             bias=epst, scale=-1.0)
        nc.scalar.activation(out=warm, in_=epst, func=Ident,
                             bias=epst, scale=epst)

        # stream: x blocks on the SP ring, two blocks per SBUF tile (the
        # matmul then runs N=256, hiding the ~213ns contended LDWEIGHTS).
        # The last two blocks are single tiles: block 15's matmuls run
        # post-stream at the uncontended rate, shortening the tail.
        xts = []  # (tile, q, first block)
        for t in range(8):
            xt = xin2.tile([P, 2, W], mmdt, tag="xt2", name=f"xt{t}")
            nc.sync.dma_start(out=xt[:, 0], in_=xr[2 * t])
            nc.sync.dma_start(out=xt[:, 1], in_=xr[2 * t + 1])
            xts.append((xt.rearrange("p q (w s) -> p q w s", s=CHUNK), 2,
                        2 * t))

        # bn_stats per PSUM bank-half-or-more (never a region the PE still
        # writes): banks 0-2 in one 512-elem call, bank 3 split 2+2 so the
        # tail only waits for the last 256-elem call
        assert nc.vector.BN_STATS_FMAX >= 4 * WP
        statsall = zpool.tile([P, 5, nc.vector.BN_STATS_DIM], f32)
        zts = [zpsum.tile([P, 4, WP], f32, tag="zt", name=f"zt{g}")
               for g in range(4)]

        for xs, q, b0 in xts:
            pt = zts[b0 // 4][:, b0 % 4:b0 % 4 + q]
            for s2 in range(CHUNK):
                nc.tensor.matmul(pt, lhsT=wtile[:, s2, :],
                                 rhs=xs[:, :, :, s2],
                                 start=(s2 == 0), stop=(s2 == CHUNK - 1))
            done = b0 + q  # blocks completed so far
            if done % 4 == 0 and done <= 12:
                g = done // 4 - 1
                nc.vector.bn_stats(out=statsall[:, g],
                                   in_=zts[g].rearrange("p a b -> p (a b)"))
            elif done == 14:
                nc.vector.bn_stats(out=statsall[:, 3],
                                   in_=zts[3][:, 0:2].rearrange(
                                       "p a b -> p (a b)"))
            elif done == 16:
                nc.vector.bn_stats(out=statsall[:, 4],
                                   in_=zts[3][:, 2:4].rearrange(
                                       "p a b -> p (a b)"))

        # Per-partition raw mean'/var over all 2048 elements; fc bias b2 is
        # folded in here (z_true = raw + b2) and into the normalize offset.
        mv = zpool.tile([P, 2], f32)
        nc.vector.bn_aggr(out=mv, in_=statsall)
        # me2 = (-mean, E2) of biased z per partition:
        #   negmean = -(mean' + b2);  E2 = negmean^2 + var
        me2 = zpool.tile([P, 2], f32)
        nc.vector.tensor_scalar(out=me2[:, 0:1], in0=mv[:, 0:1],
                                scalar1=cvt[:, 0:1], scalar2=-1.0,
                                op0=Add, op1=Mult)
        nc.vector.scalar_tensor_tensor(
            out=me2[:, 1:2], in0=me2[:, 0:1], scalar=me2[:, 0:1],
            in1=mv[:, 1:2], op0=Mult, op1=Add)
        # Grouped cross-partition average + broadcast in one matmul:
        # gp[p'] = (1/16) * sum_{p in group(p')} me2[p] = (-mean_g, E2_g)
        gp = psg.tile([P, 2], f32)
        nc.tensor.matmul(gp, lhsT=gmt, rhs=me2, start=True, stop=True)
        gsb = zpool.tile([P, 2], f32)
        nc.vector.tensor_copy(gsb, gp)
        # negvar = mean_g^2 - E2_g;  sd = sqrt(-negvar + eps)
        negvar = zpool.tile([P, 1], f32)
        nc.vector.scalar_tensor_tensor(
            out=negvar, in0=gsb[:, 0:1], scalar=gsb[:, 0:1], in1=gsb[:, 1:2],
            op0=Mult, op1=Sub)
        sd = zpool.tile([P, 1], f32)
        nc.scalar.activation(out=sd, in_=negvar, func=Sqrt, bias=epst,
                             scale=-1.0)
        # v = b2 - mean_g (overlaps with Sqrt on the ACT engine)
        v = zpool.tile([P, 1], f32)
        nc.vector.tensor_add(v, cvt[:, 0:1], gsb[:, 0:1])
        rs = zpool.tile([P, 1], f32)
        nc.vector.reciprocal(rs, sd)
        # out = raw*A + B2 with A = rsqrt*gamma, B2 = (b2 - mean_g)*A + beta
        A = zpool.tile([P, 1], f32)
        nc.vector.tensor_mul(A, rs, cvt[:, 1:2])
        B2 = zpool.tile([P, 1], f32)
        nc.vector.scalar_tensor_tensor(
            out=B2, in0=A, scalar=v, in1=cvt[:, 2:3], op0=Mult, op1=Add)

        # normalize PSUM->SBUF in tapered chunks (small first so the output
        # DMA starts early; ACT and DVE alternate), each chunk's DMA
        # overlapping the next chunk's normalize
        onorm = zpool.tile([P, RB * WP], f32)
        chunks = [(0, 1), (1, 4), (4, 8), (8, 12), (12, 16)]
        for ci, (b0, b1) in enumerate(chunks):
            sl = slice(b0 * WP, b1 * WP)
            g = b0 // 4
            zin = zts[g][:, b0 % 4:b0 % 4 + (b1 - b0)].rearrange(
                "p a b -> p (a b)")
            if ci % 2 == 0:
                nc.vector.tensor_scalar(out=onorm[:, sl], in0=zin,
                                        scalar1=A, scalar2=B2,
                                        op0=Mult, op1=Add)
            else:
                nc.scalar.activation(out=onorm[:, sl], in_=zin,
                                     func=Ident, scale=A, bias=B2)
            eng = nc.sync if ci % 2 == 0 else nc.scalar
            eng.dma_start(out=out[:, sl], in_=onorm[:, sl])

    nc.compile()
    return nc


def _host_weights(fc_wr, fc_wi, fc_br, fc_bi, conv_w, conv_b, gamma, beta):
    fc_wr = np.asarray(fc_wr, np.float64)
    fc_wi = np.asarray(fc_wi, np.float64)
    fc_br = np.asarray(fc_br, np.float64)
    fc_bi = np.asarray(fc_bi, np.float64)
    conv_w = np.asarray(conv_w, np.float64)
    conv_b = np.asarray(conv_b, np.float64)
    gamma = np.asarray(gamma, np.float64)
    beta = np.asarray(beta, np.float64)

    j = np.arange(D_IN)
    F = np.exp(-2j * np.pi * np.outer(j, j) / D_IN) / np.sqrt(D_IN)
    d = np.arange(D)
    G = np.exp(2j * np.pi * np.outer(d, d) / D) / np.sqrt(D)
    Wc = fc_wr + 1j * fc_wi
    bc = fc_br + 1j * fc_bi
    M2 = (np.real(F @ Wc.T @ G) @ conv_w.T).astype(np.float32)  # [256, 16]
    b2 = (np.real(bc @ G) @ conv_w.T + conv_b).astype(np.float32)  # [16]

    # Compact weights, contraction dim k=(par, s1) of 32:
    #   lhsT cols [0:128]: wlc[par*16+s1, p] = (p%16==s1 and (p//16)%2==par)
    #   rhs  cols [128:640] ([s2, 32-frame]): the 16x16 M2 block at
    #     sub-offset 16*par, so out[(hb,s1'), (s2,c)] lands pre-placed for
    #     the 32-aligned on-device copies.
    wlc = np.zeros((32, 640), np.float32)
    p_ = np.arange(P)
    for par in range(2):
        for s1 in range(CHUNK):
            k = par * 16 + s1
            wlc[k, :P] = ((p_ % 16 == s1) & ((p_ // 16) % 2 == par))
            for s2 in range(CHUNK):
                c0 = 128 + s2 * 32 + 16 * par
                wlc[k, c0:c0 + 16] = M2[s1 * 16 + s2]

    # Group-average + broadcast mask; each partition holds 2048 elements,
    # each group spans 16 partitions -> scale 1/16 on the per-partition means
    pidx = np.arange(P)
    grp = (pidx % D) // (D // GROUPS)
    gmask = (grp[:, None] == grp[None, :]).astype(np.float32) / 16.0

    e = pidx % D
    consts = np.stack([b2[e], gamma.astype(np.float32)[e],
                       beta.astype(np.float32)[e]], axis=1)  # [128, 3]
    return wlc, gmask, consts


def kernel(x, fc_wr, fc_wi, fc_br, fc_bi, conv_w, conv_b, gamma, beta,
           _return_results=False, _trace=False, _mm_dtype="float32r"):
    from concourse.bass_utils import run_bass_kernel_spmd

    if _mm_dtype not in _CACHED_NC:
        _CACHED_NC[_mm_dtype] = _build_nc(_mm_dtype)
    nc = _CACHED_NC[_mm_dtype]

    wlc, gmask, consts = _host_weights(fc_wr, fc_wi, fc_br, fc_bi,
                                      conv_w, conv_b, gamma, beta)
    x = np.ascontiguousarray(np.asarray(x, np.float32).reshape(B, H, W))
    in_maps = [{"x": x[b], "wlc": wlc, "gmask": gmask, "consts": consts}
               for b in range(N_CORES)]
    res = run_bass_kernel_spmd(nc, in_maps, list(range(N_CORES)),
                               trace=_trace)
    # device layout [p=(hblk,e), rg, w] -> [D, HP, WP], hi = rg*8 + hblk
    out = np.stack(
        [res.results[b]["out"].reshape(8, D, RB, WP)
         .transpose(1, 2, 0, 3).reshape(D, HP, WP)
         for b in range(N_CORES)], axis=0)
    if _return_results:
        return out, res
    return out


# revision 34
# speedup vs baseline: 1.1044x; 1.0394x over previous
"""CFNO forward kernel for Trainium2 (8 NeuronCores, data-parallel over batch).

The reference computes, per 16x16 patch p (flattened to 256):
    fft = FFT_256(p) (ortho); fc = fft @ Wc^T + bc; y = Re(IFFT_16(fc)) (ortho)
    z = y @ conv_w^T + conv_b;  out = GroupNorm_8(z) * gamma + beta

Because p is real and every step before GroupNorm is linear, the whole chain
folds into one real matrix on the host:
    M2 = Re(F @ Wc^T @ G) @ conv_w^T   [256, 16]
    b2 = Re(bc @ G) @ conv_w^T + conv_b [16]
    z  = p @ M2 + b2
(F = symmetric 256-pt DFT matrix / sqrt(256); G = inverse 16-pt DFT / sqrt(16))

On-device per core (one batch image, x [2048, 2048]):
  - 16 row-blocks of 128 image rows, streamed as 8 two-block tiles that all
    stay resident in SBUF (stream never write-after-read gated); SBUF layout
    [128 part=(hblk, s1), 2, 2048]
  - per tile, 16 PSUM-accumulating matmuls (one per patch-column offset s2,
    free dim 256 = 2 blocks x 128 patches) with a block-diagonal lhsT so
    all 8 h-blocks share a matmul; float32r keeps fp32 storage with a fast
    (TF32-like) PE mode.  N=256 matters: the fixed ~213ns LDWEIGHTS (slow
    under concurrent DMA-stream SBUF traffic) hides under a ~220ns matmul,
    while N=128 tiles run LDWEIGHTS-bound at half throughput
  - the block-diagonal lhsT is built on-chip from an 80KB compact form (one
    contraction-32 matmul pre-places each hblk's 16x16 block in a 32-wide
    frame, then 4 copies at the legal 32-aligned partition bases), instead
    of streaming the 1MB expanded matrix from HBM
  - z stays in PSUM (4 banks hold all 16 blocks); bn_stats reads PSUM once
    per bank (512 elems = FMAX) so the DVE read never stalls the PE with a
    false WAR on a bank it still accumulates into; the fc bias b2 is folded
    into the final normalize coefficients
  - one mask-matmul does the grouped cross-partition reduce AND broadcast
  - normalize (z*A + B) fused with the PSUM->SBUF move in tapered chunks
    (1/3/4/4/4 blocks), each chunk's output DMA (on alternating queues)
    overlapping the next chunk's normalize

DMA scheduling (probed on HW): each block is one uniform [128, 2048]
dma_start.  Only full 128-partition transfers run at line rate —
partition-subset dma_starts (e.g. [0:112] + tails) drop to ~half the
per-engine rate, and non-divisible partition counts like [0:127]
degenerate onto a single engine.  Engine 15 is ~17% slower than the rest,
but only under all-8-core load (cross-core HBM arbitration) — shrinking
total stream bytes (compact weights) is the only effective relief.
"""

import numpy as np
from contextlib import ExitStack

CHUNK = 16
GROUPS = 8
EPS = 1e-5
B, C, H, W = 8, 1, 2048, 2048
D = 16
D_IN = CHUNK * CHUNK * C  # 256
HP = H // CHUNK  # 128 patch rows
WP = W // CHUNK  # 128 patch cols
P = 128
RB = 16  # 128-row blocks per image
N_CORES = 8

_CACHED_NC = {}


def _build_nc(mm_dtype="float32r"):
    import concourse.bass as bass
    import concourse.tile as tile
    from concourse import bacc, mybir

    f32 = mybir.dt.float32
    mmdt = getattr(mybir.dt, mm_dtype)
    nc = bacc.Bacc("TRN2", target_bir_lowering=False, debug=False,
                   num_devices=N_CORES)

    x = nc.dram_tensor("x", [H, W], mmdt, kind="ExternalInput").ap()
    # compact weights: [32, 0:128] replication lhsT + [32, 128:640] pre-placed
    # rhs; one matmul broadcasts/places the per-hblk blocks (80KB vs 1MB)
    wlc = nc.dram_tensor("wlc", [32, 640], mmdt, kind="ExternalInput").ap()
    gmask = nc.dram_tensor("gmask", [P, P], f32, kind="ExternalInput").ap()
    consts = nc.dram_tensor("consts", [P, 3], f32, kind="ExternalInput").ap()
    # [p=(hblk,e), rg, w] flattened (rg = hi//8); host reorders to [D, HP, WP]
    out = nc.dram_tensor("out", [P, RB * WP], f32, kind="ExternalOutput").ap()

    Ident = mybir.ActivationFunctionType.Identity
    Sqrt = mybir.ActivationFunctionType.Sqrt
    Mult = mybir.AluOpType.mult
    Add = mybir.AluOpType.add
    Sub = mybir.AluOpType.subtract

    with tile.TileContext(nc) as tc, ExitStack() as ctx:
        const_pool = ctx.enter_context(tc.tile_pool(name="const", bufs=1))
        # all 16 x blocks fit in SBUF: the stream is never WAR-gated
        xin2 = ctx.enter_context(tc.tile_pool(name="xin2", bufs=8))
        zpool = ctx.enter_context(tc.tile_pool(name="z", bufs=1))
        # 4 persistent PSUM banks hold z for all 16 blocks; 1 more for gp
        zpsum = ctx.enter_context(tc.tile_pool(name="zp", bufs=4, space="PSUM"))
        psg = ctx.enter_context(tc.tile_pool(name="psg", bufs=1, space="PSUM"))

        # x row-block rb covers image rows [rb*128, (rb+1)*128):
        # row = rb*128 + p, p = (hblk, s1)
        xr = x.rearrange("(rb p) c -> rb p c", rb=RB, p=P)

        # compact weights (80KB instead of 1MB off the stream): one matmul
        # (contraction 32) broadcasts each hblk's 16x16 block, already placed
        # at sub-offset (hb%2)*16 of a 32-wide frame; then 4 copies at the
        # legal 32-aligned partition bases move it into the block-diagonal
        # lhsT.  Zeros come from an f32 staging memset (f32r memset is not a
        # valid ISA instruction).
        ws = const_pool.tile([32, 640], mmdt)
        nc.scalar.dma_start(out=ws, in_=wlc)
        gmt = const_pool.tile([P, P], f32)
        nc.scalar.dma_start(out=gmt, in_=gmask)
        cvt = const_pool.tile([P, 3], f32)
        nc.scalar.dma_start(out=cvt, in_=consts)
        wtile = const_pool.tile([P, CHUNK, P], mmdt)
        wflat = wtile.rearrange("p s m -> p (s m)")
        zstage = const_pool.tile([P, 512], f32)
        nc.vector.memset(zstage, 0.0)
        for k in range(4):
            nc.vector.tensor_copy(wflat[:, 512 * k:512 * (k + 1)], zstage)
        wexp = psg.tile([P, 512], f32, name="wexp")
        nc.tensor.matmul(wexp, lhsT=ws[:, 0:128], rhs=ws[:, 128:640],
                         start=True, stop=True)
        for k in range(4):
            nc.vector.tensor_copy(
                wtile[32 * k:32 * (k + 1), :, 32 * k:32 * (k + 1)],
                wexp[32 * k:32 * (k + 1)].rearrange("p (a b) -> p a b",
                                                    a=CHUNK))
        epst = const_pool.tile([P, 1], f32)
        nc.vector.memset(epst, EPS)
        # touch Sqrt (exact scale/bias shape used later) early so its
        # ACT table loads during the stream, not in the stats chain
        warm = const_pool.tile([P, 1], f32)
        nc.scalar.activation(out=warm, in_=epst, func=Sqrt,
                             bias=epst, scale=-1.0)
        nc.scalar.activation(out=warm, in_=epst, func=Ident,
                             bias=epst, scale=epst)

        # stream: x blocks on the SP ring, two blocks per SBUF tile (the
        # matmul then runs N=256, hiding the ~213ns contended LDWEIGHTS).
        # The last two blocks are single tiles: block 15's matmuls run
        # post-stream at the uncontended rate, shortening the tail.
        xts = []  # (tile, q, first block)
        for t in range(8):
            xt = xin2.tile([P, 2, W], mmdt, tag="xt2", name=f"xt{t}")
            nc.sync.dma_start(out=xt[:, 0], in_=xr[2 * t])
            nc.sync.dma_start(out=xt[:, 1], in_=xr[2 * t + 1])
            xts.append((xt.rearrange("p q (w s) -> p q w s", s=CHUNK), 2,
                        2 * t))

        # bn_stats per PSUM bank-half-or-more (never a region the PE still
        # writes): banks 0-2 in one 512-elem call, bank 3 split 2+2 so the
        # tail only waits for the last 256-elem call
        assert nc.vector.BN_STATS_FMAX >= 4 * WP
        statsall = zpool.tile([P, 5, nc.vector.BN_STATS_DIM], f32)
        zts = [zpsum.tile([P, 4, WP], f32, tag="zt", name=f"zt{g}")
               for g in range(4)]

        for xs, q, b0 in xts:
            pt = zts[b0 // 4][:, b0 % 4:b0 % 4 + q]
            for s2 in range(CHUNK):
                nc.tensor.matmul(pt, lhsT=wtile[:, s2, :],
                                 rhs=xs[:, :, :, s2],
                                 start=(s2 == 0), stop=(s2 == CHUNK - 1))
            done = b0 + q  # blocks completed so far
            if done % 4 == 0 and done <= 12:
                g = done // 4 - 1
                nc.vector.bn_stats(out=statsall[:, g],
                                   in_=zts[g].rearrange("p a b -> p (a b)"))
            elif done == 14:
                nc.vector.bn_stats(out=statsall[:, 3],
                                   in_=zts[3][:, 0:2].rearrange(
                                       "p a b -> p (a b)"))
            elif done == 16:
                nc.vector.bn_stats(out=statsall[:, 4],
                                   in_=zts[3][:, 2:4].rearrange(
                                       "p a b -> p (a b)"))

        # Per-partition raw mean'/var over all 2048 elements; fc bias b2 is
        # folded in here (z_true = raw + b2) and into the normalize offset.
        mv = zpool.tile([P, 2], f32)
        nc.vector.bn_aggr(out=mv, in_=statsall)
        # me2 = (-mean, E2) of biased z per partition:
        #   negmean = -(mean' + b2);  E2 = negmean^2 + var
        me2 = zpool.tile([P, 2], f32)
        nc.vector.tensor_scalar(out=me2[:, 0:1], in0=mv[:, 0:1],
                                scalar1=cvt[:, 0:1], scalar2=-1.0,
                                op0=Add, op1=Mult)
        nc.vector.scalar_tensor_tensor(
            out=me2[:, 1:2], in0=me2[:, 0:1], scalar=me2[:, 0:1],
            in1=mv[:, 1:2], op0=Mult, op1=Add)
        # Grouped cross-partition average + broadcast in one matmul:
        # gp[p'] = (1/16) * sum_{p in group(p')} me2[p] = (-mean_g, E2_g)
        gp = psg.tile([P, 2], f32)
        nc.tensor.matmul(gp, lhsT=gmt, rhs=me2, start=True, stop=True)
        gsb = zpool.tile([P, 2], f32)
        nc.vector.tensor_copy(gsb, gp)
        # negvar = mean_g^2 - E2_g;  sd = sqrt(-negvar + eps)
        negvar = zpool.tile([P, 1], f32)
        nc.vector.scalar_tensor_tensor(
            out=negvar, in0=gsb[:, 0:1], scalar=gsb[:, 0:1], in1=gsb[:, 1:2],
            op0=Mult, op1=Sub)
        sd = zpool.tile([P, 1], f32)
        nc.scalar.activation(out=sd, in_=negvar, func=Sqrt, bias=epst,
                             scale=-1.0)
        # v = b2 - mean_g (overlaps with Sqrt on the ACT engine)
        v = zpool.tile([P, 1], f32)
        nc.vector.tensor_add(v, cvt[:, 0:1], gsb[:, 0:1])
        rs = zpool.tile([P, 1], f32)
        nc.vector.reciprocal(rs, sd)
        # out = raw*A + B2 with A = rsqrt*gamma, B2 = (b2 - mean_g)*A + beta
        A = zpool.tile([P, 1], f32)
        nc.vector.tensor_mul(A, rs, cvt[:, 1:2])
        B2 = zpool.tile([P, 1], f32)
        nc.vector.scalar_tensor_tensor(
            out=B2, in0=A, scalar=v, in1=cvt[:, 2:3], op0=Mult, op1=Add)

        # normalize PSUM->SBUF in tapered chunks (small first so the output
        # DMA starts early; ACT and DVE alternate), each chunk's DMA
        # overlapping the next chunk's normalize
        onorm = zpool.tile([P, RB * WP], f32)
        chunks = [(0, 1), (1, 4), (4, 8), (8, 12), (12, 15), (15, 16)]
        for ci, (b0, b1) in enumerate(chunks):
            sl = slice(b0 * WP, b1 * WP)
            g = b0 // 4
            zin = zts[g][:, b0 % 4:b0 % 4 + (b1 - b0)].rearrange(
                "p a b -> p (a b)")
            if ci % 2 == 0:
                nc.vector.tensor_scalar(out=onorm[:, sl], in0=zin,
                                        scalar1=A, scalar2=B2,
                                        op0=Mult, op1=Add)
            else:
                nc.scalar.activation(out=onorm[:, sl], in_=zin,
                                     func=Ident, scale=A, bias=B2)
            eng = nc.sync if ci % 2 == 0 else nc.scalar
            eng.dma_start(out=out[:, sl], in_=onorm[:, sl])

    nc.compile()
    return nc


def _host_weights(fc_wr, fc_wi, fc_br, fc_bi, conv_w, conv_b, gamma, beta):
    fc_wr = np.asarray(fc_wr, np.float64)
    fc_wi = np.asarray(fc_wi, np.float64)
    fc_br = np.asarray(fc_br, np.float64)
    fc_bi = np.asarray(fc_bi, np.float64)
    conv_w = np.asarray(conv_w, np.float64)
    conv_b = np.asarray(conv_b, np.float64)
    gamma = np.asarray(gamma, np.float64)
    beta = np.asarray(beta, np.float64)

    j = np.arange(D_IN)
    F = np.exp(-2j * np.pi * np.outer(j, j) / D_IN) / np.sqrt(D_IN)
    d = np.arange(D)
    G = np.exp(2j * np.pi * np.outer(d, d) / D) / np.sqrt(D)
    Wc = fc_wr + 1j * fc_wi
    bc = fc_br + 1j * fc_bi
    M2 = (np.real(F @ Wc.T @ G) @ conv_w.T).astype(np.float32)  # [256, 16]
    b2 = (np.real(bc @ G) @ conv_w.T + conv_b).astype(np.float32)  # [16]

    # Compact weights, contraction dim k=(par, s1) of 32:
    #   lhsT cols [0:128]: wlc[par*16+s1, p] = (p%16==s1 and (p//16)%2==par)
    #   rhs  cols [128:640] ([s2, 32-frame]): the 16x16 M2 block at
    #     sub-offset 16*par, so out[(hb,s1'), (s2,c)] lands pre-placed for
    #     the 32-aligned on-device copies.
    wlc = np.zeros((32, 640), np.float32)
    p_ = np.arange(P)
    for par in range(2):
        for s1 in range(CHUNK):
            k = par * 16 + s1
            wlc[k, :P] = ((p_ % 16 == s1) & ((p_ // 16) % 2 == par))
            for s2 in range(CHUNK):
                c0 = 128 + s2 * 32 + 16 * par
                wlc[k, c0:c0 + 16] = M2[s1 * 16 + s2]

    # Group-average + broadcast mask; each partition holds 2048 elements,
    # each group spans 16 partitions -> scale 1/16 on the per-partition means
    pidx = np.arange(P)
    grp = (pidx % D) // (D // GROUPS)
    gmask = (grp[:, None] == grp[None, :]).astype(np.float32) / 16.0

    e = pidx % D
    consts = np.stack([b2[e], gamma.astype(np.float32)[e],
                       beta.astype(np.float32)[e]], axis=1)  # [128, 3]
    return wlc, gmask, consts


def kernel(x, fc_wr, fc_wi, fc_br, fc_bi, conv_w, conv_b, gamma, beta,
           _return_results=False, _trace=False, _mm_dtype="float32r"):
    from concourse.bass_utils import run_bass_kernel_spmd

    if _mm_dtype not in _CACHED_NC:
        _CACHED_NC[_mm_dtype] = _build_nc(_mm_dtype)
    nc = _CACHED_NC[_mm_dtype]

    wlc, gmask, consts = _host_weights(fc_wr, fc_wi, fc_br, fc_bi,
                                      conv_w, conv_b, gamma, beta)
    x = np.ascontiguousarray(np.asarray(x, np.float32).reshape(B, H, W))
    in_maps = [{"x": x[b], "wlc": wlc, "gmask": gmask, "consts": consts}
               for b in range(N_CORES)]
    res = run_bass_kernel_spmd(nc, in_maps, list(range(N_CORES)),
                               trace=_trace)
    # device layout [p=(hblk,e), rg, w] -> [D, HP, WP], hi = rg*8 + hblk
    out = np.stack(
        [res.results[b]["out"].reshape(8, D, RB, WP)
         .transpose(1, 2, 0, 3).reshape(D, HP, WP)
         for b in range(N_CORES)], axis=0)
    if _return_results:
        return out, res
    return out
